# revision 38
# baseline (speedup 1.0000x reference)
"""TRN2 Bass kernel for nn_MultiHeadAttention (B=4, S=2048, D=1024, H=16).

Self-contained: builds and runs an SPMD Bass/Tile program on the 8
axon-tunneled NeuronCores. Sharding: core c = (batch c//2, query-half c%2);
no collectives (each core of a batch pair recomputes the K/V projections
for its batch, which avoids any cross-core communication).

v2 design (all-fp16 operands, host-side pre-transposes):
  host supplies xqT [D,SQ], kT/vT [D,S], maskT [S,SQ] (all fp16) so the
  device does zero PE transposes and zero dtype casts.
  A : QT[hp*128, q]  = sum_dc wq[dc,m]^T @ xqT[dc, q]      (psum per m)
  B1: KT[hp*128, k]  = sum_dc wk[dc,m]^T @ kT[dc, k]
      VA[k, h*65]    = sum_dc vT[dc,kblk]^T @ wv[dc, :]    (ones col -> denom)
  B3: per head-pair hp: accumulate over ALL 16 k-chunks in PSUM:
        sT[k,q] = KhT^T @ QhT ; p = exp(sT/8)*maskT (fp16)
        o_hl[65, q] += VA^T @ p          (ones row = denominator)
      then R = e2^T @ (1/sums), UT[hp] = o * R  (fp16)
  D : out[q, :] = sum_dc UT[dc-chunk, q]^T @ wo[dc, :]
"""
import sys
sys.path.insert(0, "/opt/trn_rl_repo")

import numpy as np
import concourse.bass as bass
import concourse.mybir as mybir
import concourse.tile as tile

F32 = mybir.dt.float32
F32R = mybir.dt.float32r
F16 = mybir.dt.float16
Exp = mybir.ActivationFunctionType.Exp
Ln = mybir.ActivationFunctionType.Ln
HD = 64  # head dim (fixed)


def split_ctrl_multiwaits(nc):
    """walrus here rejects >1 sync-wait per instruction; move extras onto
    single-wait NoOps inserted before the instruction on the same engine."""
    n_fixed = 0
    for f in nc.m.functions:
        for bb in f.blocks:
            insts = bb.instructions
            i = 0
            while i < len(insts):
                ins = insts[i]
                si = ins.sync_info
                if si is not None and len(si.on_wait) > 1:
                    waits = list(si.on_wait)
                    si.on_wait = waits[-1:]
                    for j, w in enumerate(waits[:-1]):
                        nop = mybir.InstNoOp(name=f"{ins.name}-ws{j}", ins=[], outs=[])
                        nop.engine = ins.engine
                        nsi = nop.sync_info
                        if nsi is None:
                            nop.sync_info = mybir.SyncInfo(on_wait=[w], on_update=[])
                        else:
                            nsi.on_wait = [w]
                        insts.insert(i, nop)
                        i += 1
                    n_fixed += 1
                i += 1
    return n_fixed


def build_mha(S, D, H, SQ, attn_dt=None, phases=None, repeat=1):
    assert D == H * HD
    HP = H // 2            # head pairs (128 dims each)
    DC = D // 128          # contraction chunks
    KC = S // 128          # k 128-chunks
    KC4 = S // 512         # k 512-chunks
    VW = HD + 1            # V_aug columns per head

    nc = bass.Bass("TRN2", target_bir_lowering=False, debug=False, num_devices=8)
    xqT = nc.dram_tensor("xqT", [D, SQ], F16, kind="ExternalInput").ap()
    ktd = nc.dram_tensor("ktd", [D, S], F16, kind="ExternalInput").ap()
    vtd = nc.dram_tensor("vtd", [D, S], F16, kind="ExternalInput").ap()
    mtd = nc.dram_tensor("mtd", [S, SQ], F16, kind="ExternalInput").ap()
    wqd = nc.dram_tensor("wqd", [D, D], F16, kind="ExternalInput").ap()
    wkd = nc.dram_tensor("wkd", [D, D], F16, kind="ExternalInput").ap()
    wvd = nc.dram_tensor("wvd", [D, D], F16, kind="ExternalInput").ap()
    wod = nc.dram_tensor("wod", [D, D], F16, kind="ExternalInput").ap()
    e2d = nc.dram_tensor("e2d", [33, 128], F16, kind="ExternalInput").ap()
    out = nc.dram_tensor("out", [SQ, D], F32, kind="ExternalOutput").ap()

    with tile.TileContext(nc) as tc:
      with (
          tc.tile_pool(name="persist", bufs=1) as persist,
      ):
        for _rep in range(repeat):
            QT = persist.tile([128, HP * SQ], F16, tag="QT")
            KT = persist.tile([128, HP * S], F16, tag="KT")
            VA = persist.tile([128, KC * H * VW], F16, tag="VA")
            maskT = persist.tile([128, KC * SQ], F16, tag="maskT")
            UT = persist.tile([128, HP * SQ], F16, tag="UT")
            e2 = persist.tile([33, 128], F16, tag="e2")
            nc.sync.dma_start(e2[:], e2d[:])

            # ---------------- Phase A: Q projection ----------------
            with (
                tc.tile_pool(name="pha", bufs=1) as pha,
                tc.tile_pool(name="psA", bufs=1, space="PSUM") as psA,
            ):
                wq = pha.tile([128, DC * D], F16, tag="wq")
                for dc in range(DC):
                    nc.sync.dma_start(
                        wq[:, dc * D:(dc + 1) * D], wqd[dc * 128:(dc + 1) * 128, :])
                xq = pha.tile([128, DC * SQ], F16, tag="xq")
                for dc in range(DC):
                    nc.gpsimd.dma_start(
                        xq[:, dc * SQ:(dc + 1) * SQ], xqT[dc * 128:(dc + 1) * 128, :])
                # V_aug ones columns (needed only from B1-V onward)
                nc.gpsimd.memset(
                    VA.rearrange("p (c x) -> p c x", x=VW)[:, :, HD:HD + 1], 1.0)

                # PE warm-up: keep the tensor engine busy while the first
                # DMAs land so the HAM clock gate reaches 2.4 GHz before
                # real work starts (and stays there).
                wup = pha.tile([128, 512], F16, tag="wup")
                nc.gpsimd.memset(wup[:], 0.0)
                wps = psA.tile([128, 512], F32, tag="m0", name="wups")
                for i in range(64):
                    nc.tensor.matmul(
                        wps[:], wup[:, 0:128], wup[:], start=True, stop=True)

                for qh in range(SQ // 512):
                    for m in range(DC):
                        ps = psA.tile([128, 512], F32, tag=f"m{m % 3}",
                                      name=f"qps{m}_{qh}")
                        for dc in range(DC):
                            nc.tensor.matmul(
                                ps[:],
                                wq[:, dc * D + m * 128: dc * D + (m + 1) * 128],
                                xq[:, dc * SQ + qh * 512: dc * SQ + qh * 512 + 512],
                                start=(dc == 0), stop=(dc == DC - 1))
                        dst = QT[:, m * SQ + qh * 512: m * SQ + qh * 512 + 512]
                        if m % 2 == 0:
                            nc.scalar.copy(dst, ps[:])
                        else:
                            nc.vector.tensor_copy(dst, ps[:])

            # ---------------- Phase B1: K and V projections ----------------
            with (
                tc.tile_pool(name="phb", bufs=1) as phb,
                tc.tile_pool(name="phb2", bufs=2) as phb2,
            ):
                wk = phb.tile([128, DC * D], F16, tag="wk")
                wv = phb.tile([128, DC * D], F16, tag="wv")
                kts0 = phb2.tile([128, DC * 512], F16, tag="kvs", name="kts0")
                for dc in range(DC):
                    nc.sync.dma_start(
                        wk[:, dc * D:(dc + 1) * D], wkd[dc * 128:(dc + 1) * 128, :])
                    nc.sync.dma_start(
                        kts0[:, dc * 512:(dc + 1) * 512],
                        ktd[dc * 128:(dc + 1) * 128, 0:512])
                for dc in range(DC):
                    nc.sync.dma_start(
                        wv[:, dc * D:(dc + 1) * D], wvd[dc * 128:(dc + 1) * 128, :])
                # prefetch maskT (used in B3) on the scalar DGE queue
                for kc in range(KC):
                    nc.scalar.dma_start(
                        maskT[:, kc * SQ:(kc + 1) * SQ],
                        mtd[kc * 128:(kc + 1) * 128, :])
                # prefetch the first vT quarter so phase B1-V starts instantly
                vts0 = phb.tile([128, DC * 512], F16, tag="vts0")
                for dc in range(DC):
                    nc.sync.dma_start(
                        vts0[:, dc * 512:(dc + 1) * 512],
                        vtd[dc * 128:(dc + 1) * 128, 0:512])
                # K projection: per 512-key chunk, all head-pairs in psum
                with tc.tile_pool(name="psK", bufs=1, space="PSUM") as psK:
                  for k4 in range(KC4):
                    if k4 == 0:
                        kts = kts0
                    else:
                        kts = phb2.tile([128, DC * 512], F16, tag="kvs",
                                        name=f"kts{k4}")
                        for dc in range(DC):
                            nc.sync.dma_start(
                                kts[:, dc * 512:(dc + 1) * 512],
                                ktd[dc * 128:(dc + 1) * 128, k4 * 512:(k4 + 1) * 512])
                    for m in range(DC):
                        ps = psK.tile([128, 512], F32, tag=f"m{m % 3}",
                                      name=f"kps{m}_{k4}")
                        for dc in range(DC):
                            nc.tensor.matmul(
                                ps[:],
                                wk[:, dc * D + m * 128: dc * D + (m + 1) * 128],
                                kts[:, dc * 512:(dc + 1) * 512],
                                start=(dc == 0), stop=(dc == DC - 1))
                        dst = KT[:, m * S + k4 * 512: m * S + k4 * 512 + 512]
                        if m % 2 == 0:
                            nc.scalar.copy(dst, ps[:])
                        else:
                            nc.vector.tensor_copy(dst, ps[:])

                # V projection: per 128-key block, k on partitions, douts free
                with tc.tile_pool(name="psV", bufs=2, space="PSUM") as psV:
                  for k4 in range(KC4):
                    if k4 == 0:
                        vts = vts0
                    else:
                        vts = phb2.tile([128, DC * 512], F16, tag="kvs",
                                        name=f"vts{k4}")
                        for dc in range(DC):
                            nc.sync.dma_start(
                                vts[:, dc * 512:(dc + 1) * 512],
                                vtd[dc * 128:(dc + 1) * 128, k4 * 512:(k4 + 1) * 512])
                    for kb in range(4):
                        kc = k4 * 4 + kb
                        ps = psV.tile([128, 1024], F32, tag="v")
                        for dc in range(DC):
                            nc.tensor.matmul(
                                ps[:, 0:512],
                                vts[:, dc * 512 + kb * 128: dc * 512 + (kb + 1) * 128],
                                wv[:, dc * D: dc * D + 512],
                                start=(dc == 0), stop=(dc == DC - 1))
                        for dc in range(DC):
                            nc.tensor.matmul(
                                ps[:, 512:1024],
                                vts[:, dc * 512 + kb * 128: dc * 512 + (kb + 1) * 128],
                                wv[:, dc * D + 512: dc * D + 1024],
                                start=(dc == 0), stop=(dc == DC - 1))
                        dst = VA[:, kc * H * VW: (kc + 1) * H * VW]
                        dst = dst.rearrange("p (h x) -> p h x", x=VW)[:, :, 0:HD]
                        src = ps.rearrange("p (h x) -> p h x", x=HD)
                        if kb % 2 == 0:
                            nc.scalar.copy(dst, src)
                        else:
                            nc.vector.tensor_copy(dst, src)

            # ---------------- Phase B3: attention, head-pair outer ----------
            # Normalization is software-pipelined: head-pair hp's o tiles are
            # drained from PSUM to SBUF (oS) immediately after the AV
            # accumulation, then the reciprocal/R/UT chain for hp is emitted
            # in the middle of hp+1's score loop so the PE never waits on it.
            with (
                tc.tile_pool(name="pp", bufs=6) as pp,
                tc.tile_pool(name="ph3", bufs=2) as ph3,
                tc.tile_pool(name="poS", bufs=2) as poS,
                tc.tile_pool(name="psS", bufs=2, space="PSUM") as psS,
                tc.tile_pool(name="psO", bufs=1, space="PSUM") as psO,
            ):
                # prefetch Wo now; the sync DGE queue is idle during B3 and
                # phase D needs it immediately.
                wo = persist.tile([128, DC * D], F16, tag="wo")
                for dc in range(DC):
                    nc.sync.dma_start(
                        wo[:, dc * D:(dc + 1) * D], wod[dc * 128:(dc + 1) * 128, :])

                pending = []

                def flush_pending():
                    if not pending:
                        return
                    php, poSt, precips = pending.pop()
                    R = psS.tile([128, SQ], F32, tag="s", name=f"R{php}")
                    for qs in range(SQ // 512):
                        nc.tensor.matmul(
                            R[:, qs * 512:(qs + 1) * 512],
                            e2[:], precips[:, qs * 512:(qs + 1) * 512],
                            start=True, stop=True)
                    nc.vector.tensor_mul(
                        UT[:, php * SQ:(php + 1) * SQ], poSt[:], R[:])

                for hp in range(HP):
                    os_ = [psO.tile([128, SQ], F32, tag=f"o{hl}", name=f"o{hl}_{hp}")
                           for hl in range(2)]
                    def emit_av(hl, p, kc):
                        h = 2 * hp + hl
                        for qs in range(SQ // 512):
                            nc.tensor.matmul(
                                os_[hl][0:VW, qs * 512:(qs + 1) * 512],
                                VA[:, kc * H * VW + h * VW: kc * H * VW + (h + 1) * VW],
                                p[:, qs * 512:(qs + 1) * 512],
                                start=(kc == 0), stop=(kc == KC - 1))

                    prev = []
                    for kc in range(KC):
                        cur = []
                        for hl in range(2):
                            poff = hl * 64
                            sT = psS.tile([128, SQ], F32, tag="s")
                            for qs in range(SQ // 512):
                                nc.tensor.matmul(
                                    sT[:, qs * 512:(qs + 1) * 512],
                                    KT[poff:poff + 64,
                                       hp * S + kc * 128: hp * S + (kc + 1) * 128],
                                    QT[poff:poff + 64,
                                       hp * SQ + qs * 512: hp * SQ + qs * 512 + 512],
                                    start=True, stop=True)
                            p = pp.tile([128, SQ], F16, tag="p")
                            nc.scalar.activation(p[:], sT[:], Exp, scale=0.125)
                            meng = nc.vector
                            meng.tensor_mul(
                                p[:], p[:], maskT[:, kc * SQ:(kc + 1) * SQ])
                            cur.append((hl, p, kc))
                        # AV delayed two k-chunks: p is long ready, and at the
                        # head-pair boundary the PSUM o drain gets extra slack.
                        if len(prev) == 2:
                            for a in prev.pop(0):
                                emit_av(*a)
                        prev.append(cur)
                        if kc == 4:
                            flush_pending()  # prev hp's R/UT, off PE critical path
                    for blk in prev:
                        for a in blk:
                            emit_av(*a)
                    # drain o to SBUF on the two idle-ish engines, freeing PSUM
                    oS = poS.tile([128, SQ], F32, tag="oS", name=f"oS_{hp}")
                    nc.vector.tensor_copy(oS[0:HD, :], os_[0][0:HD, :])
                    nc.vector.tensor_copy(oS[64:64 + HD, :], os_[1][0:HD, :])
                    sums33 = ph3.tile([33, SQ], F32, tag="sums33", name=f"su{hp}")
                    nc.gpsimd.memset(sums33[0:32, :], 1.0)
                    nc.vector.tensor_copy(sums33[0:1, :], os_[0][HD:HD + 1, :])
                    nc.vector.tensor_copy(sums33[32:33, :], os_[1][HD:HD + 1, :])
                    # 1/x as exp(-ln(x)) on the scalar engine: fast, off the
                    # PE critical path, and its f32r output satisfies the
                    # fp32r-matmul rounding requirement.
                    lnz = ph3.tile([33, SQ], F32, tag="lnz", name=f"ln{hp}")
                    nc.scalar.activation(lnz[:], sums33[:], Ln)
                    recips33 = ph3.tile([33, SQ], F32R, tag="recips33", name=f"re{hp}")
                    nc.scalar.activation(recips33[:], lnz[:], Exp, scale=-1.0)
                    pending.append((hp, oS, recips33))
                flush_pending()

            # ---------------- Phase D: out = UT^T @ Wo ----------------
            with (
                tc.tile_pool(name="phd", bufs=1) as phd,
                tc.tile_pool(name="phd2", bufs=2) as phd2,
                tc.tile_pool(name="psD", bufs=2, space="PSUM") as psD,
            ):
                for qt in range(SQ // 128):
                    ps = psD.tile([128, D], F32, tag="d")
                    for dc in range(DC):
                        for j in range(2):
                            nc.tensor.matmul(
                                ps[:, j * 512:(j + 1) * 512],
                                UT[:, dc * SQ + qt * 128: dc * SQ + (qt + 1) * 128],
                                wo[:, dc * D + j * 512: dc * D + (j + 1) * 512],
                                start=(dc == 0), stop=(dc == DC - 1))
                    ot = phd2.tile([128, D], F32, tag="ot")
                    if qt % 2 == 0:
                        nc.scalar.copy(ot[:], ps[:])
                    else:
                        nc.vector.tensor_copy(ot[:], ps[:])
                    nc.sync.dma_start(out[qt * 128:(qt + 1) * 128, :], ot[:])

    return nc


"""Shared runner: execute a Bass program on the 8 axon-tunneled NeuronCores
via bass2jax, with support for repeated calls (steady-state wall timing)."""
import time
import jax
from jax.sharding import Mesh, PartitionSpec
from jax.experimental.shard_map import shard_map

from concourse import bass2jax
from concourse.bass2jax import _bass_exec_p, install_neuronx_cc_hook, partition_id_tensor


class SpmdRunner:
    def __init__(self, nc, n_cores):
        install_neuronx_cc_hook()
        self.nc = nc
        self.n_cores = n_cores
        partition_name = nc.partition_id_tensor.name if nc.partition_id_tensor else None
        in_names, out_names, out_avals = [], [], []
        for alloc in nc.m.functions[0].allocations:
            if not isinstance(alloc, mybir.MemoryLocationSet):
                continue
            name = alloc.memorylocations[0].name
            if alloc.kind == "ExternalInput":
                if name != partition_name:
                    in_names.append(name)
            elif alloc.kind == "ExternalOutput":
                out_names.append(name)
                shape = tuple(alloc.tensor_shape)
                dtype = mybir.dt.np(alloc.dtype)
                out_avals.append(jax.core.ShapedArray(shape, dtype))
        self.in_names, self.out_names, self.out_avals = in_names, out_names, out_avals
        n_params = len(in_names)
        all_names = list(in_names) + list(out_names)
        if partition_name is not None:
            all_names.append(partition_name)

        def _body(*args):
            operands = list(args)
            if partition_name is not None:
                operands.append(partition_id_tensor())
            outs = _bass_exec_p.bind(
                *operands,
                out_avals=tuple(out_avals),
                in_names=tuple(all_names),
                out_names=tuple(out_names),
                lowering_input_output_aliases=(),
                sim_require_finite=True,
                sim_require_nnan=True,
                nc=nc,
            )
            return tuple(outs)

        devices = jax.devices()[:n_cores]
        self.mesh = Mesh(np.asarray(devices), ("core",))
        in_specs = (PartitionSpec("core"),) * (n_params + len(out_names))
        out_specs = (PartitionSpec("core"),) * len(out_names)
        # no donation: our kernels write every output element, so uninit
        # output buffers are fine and we can re-run without re-staging.
        self.fn = jax.jit(
            shard_map(_body, mesh=self.mesh, in_specs=in_specs,
                      out_specs=out_specs, check_rep=False),
            keep_unused=True,
        )
        self.n_params = n_params

    def stage(self, in_maps):
        """Concatenate per-core inputs and device_put once."""
        n = self.n_cores
        assert len(in_maps) == n
        concat_in = [
            np.concatenate([np.asarray(in_maps[c][name]) for c in range(n)], axis=0)
            for name in self.in_names
        ]
        concat_zeros = [
            np.zeros((n * a.shape[0], *a.shape[1:]), a.dtype) for a in self.out_avals
        ]
        self.args = [jax.device_put(a) for a in concat_in + concat_zeros]
        return self

    def run(self):
        outs = self.fn(*self.args)
        jax.block_until_ready(outs)
        return outs

    def results(self, outs):
        n = self.n_cores
        return [
            {
                name: np.asarray(outs[i]).reshape(n, *self.out_avals[i].shape)[c]
                for i, name in enumerate(self.out_names)
            }
            for c in range(n)
        ]

    def time_runs(self, iters=10, warmup=2):
        for _ in range(warmup):
            self.run()
        ts = []
        for _ in range(iters):
            t0 = time.perf_counter()
            self.run()
            ts.append(time.perf_counter() - t0)
        return min(ts), float(np.median(ts)), max(ts)

    def _run_batch(self, m):
        outs = None
        t0 = time.perf_counter()
        for _ in range(m):
            outs = self.fn(*self.args)
        jax.block_until_ready(outs)
        return time.perf_counter() - t0

    def time_async(self, m1=4, m2=36, reps=6):
        """Pipelined-dispatch timing: per-exec ~= (wall(m2)-wall(m1))/(m2-m1)."""
        self.run()
        w1 = min(self._run_batch(m1) for _ in range(reps))
        w2 = min(self._run_batch(m2) for _ in range(reps))
        return (w2 - w1) / (m2 - m1), w1, w2


# ----------------------------------------------------------------------------
# Host-side entry: shard full inputs over the 8 NeuronCores, run, gather.
# ----------------------------------------------------------------------------
B, S, D, H = 4, 2048, 1024, 16
SQ = S // 2
NCORES = 8

_runner_cache = []


def _get_runner():
    if not _runner_cache:
        nc = build_mha(S, D, H, SQ)
        split_ctrl_multiwaits(nc)
        _runner_cache.append(SpmdRunner(nc, NCORES))
    return _runner_cache[0]


def _make_in_maps(q, k, v, mask, Wq, Wk, Wv, Wo):
    e2 = np.zeros((33, 128), np.float16)
    e2[0, 0:64] = 1.0
    e2[32, 64:128] = 1.0
    wq16 = Wq.astype(np.float16)
    wk16 = Wk.astype(np.float16)
    wv16 = Wv.astype(np.float16)
    wo16 = Wo.astype(np.float16)
    kts = [np.ascontiguousarray(k[b].T.astype(np.float16)) for b in range(B)]
    vts = [np.ascontiguousarray(v[b].T.astype(np.float16)) for b in range(B)]
    in_maps = []
    for c in range(NCORES):
        b, qh = c // 2, c % 2
        in_maps.append({
            "xqT": np.ascontiguousarray(
                q[b, qh * SQ:(qh + 1) * SQ].T.astype(np.float16)),
            "ktd": kts[b],
            "vtd": vts[b],
            "mtd": np.ascontiguousarray(
                mask[b, qh * SQ:(qh + 1) * SQ].T.astype(np.float16)),
            "wqd": wq16, "wkd": wk16, "wvd": wv16, "wod": wo16, "e2d": e2,
        })
    return in_maps


def kernel(q, k, v, mask, Wq, Wk, Wv, Wo):
    q = np.asarray(q, np.float32)
    k = np.asarray(k, np.float32)
    v = np.asarray(v, np.float32)
    mask = np.asarray(mask, np.int32)
    Wq, Wk, Wv, Wo = (np.asarray(a, np.float32) for a in (Wq, Wk, Wv, Wo))
    r = _get_runner()
    r.stage(_make_in_maps(q, k, v, mask, Wq, Wk, Wv, Wo))
    res = r.results(r.run())
    out = np.empty((B, S, D), np.float32)
    for c in range(NCORES):
        b, qh = c // 2, c % 2
        out[b, qh * SQ:(qh + 1) * SQ] = res[c]["out"]
    return out
